# revision 1
# baseline (speedup 1.0000x reference)
"""Trainium2 Bass kernel for nn_CPCModel (CPC-style NCE loss).

Strategy (8 NeuronCores, full inputs on every core, no collectives):

The reference's leave-one-out softmax pooling collapses algebraically:
    pooled[i] = (T - e_i * zt_i) / (S - e_i),  e = exp(s), S = sum(e), T = sum(e_j zt_j)
so the [B,B] pooling matrix is never materialized.  The loss needs only
    nce = -mean_i( total[i,i] - logsumexp_j total[i,j] )
with  total[i, j in group g] = Azw_g[i]·pooled_g[j] + Czw[i]·c[j] + delta_g[i]
where Azw_g = zw @ Ww_g, Czw = zw @ Wk_w, delta_g = zw @ (Ww_g_b + Wk_b).

Each core redundantly computes the cheap pooling prep for all 4096 rows
(no collectives) and computes its own 512 rows of the [4096,4096] total
matrix + row-wise sum(exp(total - 44)); the diagonal comes from an
elementwise product.  Host sums 8x[128,4] partial row values.

Dtypes: the big matmuls (U = [Czw;Azw_g] builds and the 512x4096 total)
run fp32r (full-rate, ~19-bit mantissa).  The small prep matmuls (zt, h,
s, broadcasts, delta, diag partition-sums) run bf16 — the fp32r ISA mode
requires 128 output partitions and even N, which those shapes violate.
Host does layout prep only (transposes / stacking of weights + zw/c).
"""

import numpy as np

import concourse.bacc as bacc
import concourse.bass as bass
import concourse.mybir as mybir
import concourse.tile as tile
from concourse.bass_utils import run_bass_kernel_spmd

N_CORES = 8
B = 4096
OWN = B // N_CORES            # 512 rows of `total` per core
G = 2048                      # group size
F32 = mybir.dt.float32
F32R = mybir.dt.float32r
BF16 = mybir.dt.bfloat16
AF = mybir.ActivationFunctionType
ALU = mybir.AluOpType
SHIFT = 44.0


def _r(ap):
    return ap.bitcast(F32R)


def _build_program(static_diag=False):
    nc = bacc.Bacc(
        "TRN2",
        target_bir_lowering=False,
        debug=False,
        num_devices=N_CORES,
    )

    def din(name, shape, dt):
        return nc.dram_tensor(name, shape, dt, kind="ExternalInput").ap()

    zwTb_d = din("zwTb", [128, B], BF16)     # concat(zw_0,zw_1).T in bf16
    zwoT_d = din("zwoT", [128, OWN], F32R)   # own 512 rows of zw, transposed
    zwoTb_d = din("zwoTb", [128, OWN], BF16)
    cT_d = din("cT", [64, B], F32R)          # c.T
    uw0_d = din("UW0", [128, 128], F32R)     # hstack(Wk_w, Ww0_w)
    uw1_d = din("UW1", [128, 128], F32R)     # hstack(Wk_w, Ww1_w)
    uwo_d = din("UWo", [128, 128], F32R)     # hstack(Wk_w, Ww_{g(core)})
    lwT0_d = din("lwT0", [128, 64], BF16)    # lin0_w.T
    lwT1_d = din("lwT1", [128, 64], BF16)    # lin1_w.T
    a1wB_d = din("a1wB", [128, 64], BF16)    # blockdiag(a0_1w.T, a1_1w.T)
    a2wB_d = din("a2wB", [64, 2], BF16)      # blockdiag(a0_2w.T, a1_2w.T)
    b0_d = din("b0", [128, 1], BF16)         # Ww0_b + Wk_b
    b1_d = din("b1", [128, 1], BF16)         # Ww1_b + Wk_b
    bo_d = din("b_own", [128, 1], BF16)      # b_{group(core)}
    sel2_d = din("sel2", [2, 128], BF16)     # [[1]*64+[0]*64, [0]*64+[1]*64]
    ones_d = din("ones", [128, 1], BF16)
    linb2_d = din("linb2", [128, 1], F32)    # [lin0_b ; lin1_b]
    a1b2_d = din("a1b2", [64, 1], F32)       # [a0_1b ; a1_1b]
    v_d = nc.dram_tensor("v", [128, 4], F32, kind="ExternalOutput").ap()

    from contextlib import ExitStack
    with tile.TileContext(nc) as tc, ExitStack() as ctx:
        pers = ctx.enter_context(tc.tile_pool(name="pers", bufs=1))
        scr = ctx.enter_context(tc.tile_pool(name="scr", bufs=2))
        pbig = ctx.enter_context(tc.tile_pool(name="pbig", bufs=2, space="PSUM"))
        psml = ctx.enter_context(tc.tile_pool(name="psml", bufs=3, space="PSUM"))
        ptin = ctx.enter_context(tc.tile_pool(name="ptin", bufs=1, space="PSUM"))

        def load(name, shape, src, dt):
            t = pers.tile(shape, dt, tag=name, name=name)
            nc.sync.dma_start(t[:], src[:])
            return t

        zwTb = load("zwTb", [128, B], zwTb_d, BF16)
        zwoT = load("zwoT", [128, OWN], zwoT_d, F32R)
        zwoTb = load("zwoTb", [128, OWN], zwoTb_d, BF16)
        uw0_s = load("uw0_s", [128, 128], uw0_d, F32R)
        uw1_s = load("uw1_s", [128, 128], uw1_d, F32R)
        uwo_s = load("uwo_s", [128, 128], uwo_d, F32R)
        lwT0 = load("lwT0", [128, 64], lwT0_d, BF16)
        lwT1 = load("lwT1", [128, 64], lwT1_d, BF16)
        lwT = [lwT0, lwT1]
        a1wB = load("a1wB", [128, 64], a1wB_d, BF16)
        a2wB = load("a2wB", [64, 2], a2wB_d, BF16)
        b0_s = load("b0_s", [128, 1], b0_d, BF16)
        b1_s = load("b1_s", [128, 1], b1_d, BF16)
        bo_s = load("bo_s", [128, 1], bo_d, BF16)
        sel2 = load("sel2", [2, 128], sel2_d, BF16)
        ones = load("ones", [128, 1], ones_d, BF16)
        linb2 = load("linb2", [128, 1], linb2_d, F32)
        a1b2 = load("a1b2", [64, 1], a1b2_d, F32)

        # ---------- V [128, 4096]: rows 0:64 = cT (direct), 64:128 = pooledT ----------
        V = pers.tile([128, B], F32R, tag="V")
        nc.sync.dma_start(V[0:64, :], cT_d[:])

        # ---------- U_g = [Czw ; Azw_g] via one stacked-weight matmul each ----------
        U0 = pers.tile([128, OWN], F32R, tag="U0")
        U1 = pers.tile([128, OWN], F32R, tag="U1")
        UOwn = pers.tile([128, OWN], F32R, tag="UOwn")
        for U, uw in [(U0, uw0_s), (U1, uw1_s), (UOwn, uwo_s)]:
            pu = psml.tile([128, 512], F32, tag="ps")
            nc.tensor.matmul(pu[:], uw[:], zwoT[:], start=True, stop=True)
            nc.vector.tensor_copy(U[:], pu[:])

        # ---------- delta bias columns: biasS[:, g*4+ic] = zw_own[ic]·b_g - SHIFT ----------
        biasS = pers.tile([128, 8], F32, tag="biasS")
        for g, bg in enumerate([b0_s, b1_s]):
            for ic in range(4):
                pd = ptin.tile([128, 1], F32, tag="pt")
                nc.tensor.matmul(pd[:], zwoTb[:, ic * 128:(ic + 1) * 128], bg[:],
                                 start=True, stop=True)
                nc.scalar.activation(biasS[:, g * 4 + ic:g * 4 + ic + 1], pd[:],
                                     AF.Copy, bias=-SHIFT)

        # ---------- ztT2 [128, 2048] bf16: zt0T on 0:64, zt1T on 64:128 ----------
        ztT2 = pers.tile([128, G], BF16, tag="ztT2")
        for ch in range(4):
            pz = psml.tile([128, 512], F32, tag="ps")
            sl = slice(ch * 512, (ch + 1) * 512)
            nc.tensor.matmul(pz[0:64, :], lwT[0][:], zwTb[:, sl],
                             start=True, stop=True)
            nc.tensor.matmul(pz[64:128, :], lwT[1][:],
                             zwTb[:, G + ch * 512:G + (ch + 1) * 512],
                             start=True, stop=True)
            # relu(x + bias) on DVE: (psum add linb2) max 0
            nc.vector.tensor_scalar(ztT2[:, sl], pz[:], linb2[:], 0.0,
                                    op0=ALU.add, op1=ALU.max)

        # ---------- hT2 [64, 2048] bf16: tanh(zt @ a1w.T + b), block-diag ----------
        hT2 = pers.tile([64, G], BF16, tag="hT2")
        for ch in range(4):
            ph = psml.tile([128, 512], F32, tag="ps")
            sl = slice(ch * 512, (ch + 1) * 512)
            nc.tensor.matmul(ph[0:64, :], a1wB[:], ztT2[:, sl],
                             start=True, stop=True)
            nc.scalar.activation(hT2[:, sl], ph[0:64, :], AF.Tanh, bias=a1b2[:])

        # ---------- scores -> eT2 [2, 2048] bf16, S2 [2,1] f32 ----------
        eT2 = pers.tile([2, G], BF16, tag="eT2")
        Sacc = pers.tile([2, 4], F32, tag="Sacc")
        for ch in range(4):
            ps_ = psml.tile([128, 512], F32, tag="ps")
            sl = slice(ch * 512, (ch + 1) * 512)
            nc.tensor.matmul(ps_[0:2, :], a2wB[:], hT2[:, sl],
                             start=True, stop=True)
            nc.scalar.activation(eT2[:, sl], ps_[0:2, :], AF.Exp,
                                 accum_out=Sacc[:, ch:ch + 1])
        S2 = pers.tile([2, 1], F32, tag="S2")
        nc.vector.reduce_sum(S2[:], Sacc[:], axis=mybir.AxisListType.X)

        # ---------- betaT2 = 1/(e - S)  (= -1/(S - e)) ----------
        bT2a = pers.tile([2, G], F32, tag="bT2a")
        nc.vector.tensor_scalar(bT2a[:], eT2[:], S2[:], None, op0=ALU.subtract)
        bT2 = pers.tile([2, G], BF16, tag="bT2")
        with nc.allow_low_precision(reason="beta in bf16 for PE outer-product"):
            nc.vector.reciprocal(bT2[:], bT2a[:])

        # ---------- ztw = zt * e_bcast (ttr also accumulates T), then pooled ----------
        ztwT2 = pers.tile([128, G], F32, tag="ztwT2")
        Tacc = pers.tile([128, 4], F32, tag="Tacc")
        for ch in range(4):
            sl = slice(ch * 512, (ch + 1) * 512)
            peb = psml.tile([128, 512], F32, tag="ps")
            nc.tensor.matmul(peb[:], sel2[:], eT2[:, sl], start=True, stop=True)
            nc.vector.tensor_tensor(ztwT2[:, sl], ztT2[:, sl], peb[:],
                                    op=ALU.mult)
            nc.vector.reduce_sum(Tacc[:, ch:ch + 1], ztwT2[:, sl],
                                 axis=mybir.AxisListType.X)
        T2 = pers.tile([128, 1], F32, tag="T2")
        nc.vector.reduce_sum(T2[:], Tacc[:], axis=mybir.AxisListType.X)

        # pooled = (ztw - T) * beta_bcast   (beta = -1/(S-e) so signs cancel)
        pooled2 = pers.tile([128, G], F32, tag="pooled2")
        for ch in range(4):
            sl = slice(ch * 512, (ch + 1) * 512)
            pbb = psml.tile([128, 512], F32, tag="ps")
            nc.tensor.matmul(pbb[:], sel2[:], bT2[:, sl], start=True, stop=True)
            nc.vector.scalar_tensor_tensor(
                out=pooled2[:, sl], in0=ztwT2[:, sl], scalar=T2[:], in1=pbb[:],
                op0=ALU.subtract, op1=ALU.mult)

        # V rows 64:128: group1 pooled at cols 2048:4096 (converting copy),
        # group0 via partition-shifting sbuf->sbuf DMA (bit-identical f32).
        nc.vector.tensor_copy(V[64:128, G:B], pooled2[64:128, :])
        nc.sync.dma_start(V[64:128, 0:G], _r(pooled2[0:64, :]))

        # ---------- main loop: total rows (own 512) x all 4096 cols ----------
        seacc = pers.tile([128, 16], F32, tag="seacc")
        for ic in range(4):
            usl = slice(ic * 128, (ic + 1) * 128)
            for pair in range(4):
                g = pair // 2
                U = U0 if g == 0 else U1
                pm = pbig.tile([128, 1024], F32, tag="pb")
                for half in range(2):
                    jt = pair * 2 + half
                    nc.tensor.matmul(
                        pm[:, half * 512:(half + 1) * 512],
                        U[:, usl],
                        V[:, jt * 512:(jt + 1) * 512],
                        start=True, stop=True)
                es = scr.tile([128, 1024], BF16, tag="escr")
                nc.scalar.activation(
                    es[:], pm[:], AF.Exp,
                    bias=biasS[:, g * 4 + ic:g * 4 + ic + 1],
                    accum_out=seacc[:, ic * 4 + pair:ic * 4 + pair + 1])

        seall = pers.tile([128, 4], F32, tag="seall")
        for ic in range(4):
            nc.vector.reduce_sum(seall[:, ic:ic + 1], seacc[:, ic * 4:(ic + 1) * 4],
                                 axis=mybir.AxisListType.X)
        lnall = pers.tile([128, 4], F32, tag="lnall")
        nc.scalar.activation(lnall[:], seall[:], AF.Ln)

        # ---------- diagonal: diag[i] = UOwn[:,i]·V[:,own_pos(i)] ----------
        if static_diag:
            vsl = slice(0, OWN)
        else:
            pid = nc.vector.partition_id()
            vsl = bass.ts(pid, OWN)
        prod2 = pers.tile([128, OWN], BF16, tag="prod2")
        nc.vector.tensor_tensor(prod2[:], UOwn[:].bitcast(F32),
                                V[:, vsl].bitcast(F32), op=ALU.mult)

        vall = pers.tile([128, 4], F32, tag="vall")
        for ic in range(4):
            pdg = ptin.tile([128, 1], F32, tag="pt")
            nc.tensor.matmul(pdg[:], prod2[:, ic * 128:(ic + 1) * 128], ones[:],
                             start=True, stop=False)
            nc.tensor.matmul(pdg[:], zwoTb[:, ic * 128:(ic + 1) * 128], bo_s[:],
                             start=False, stop=True)
            # v = (diag_raw + delta - 44) - ln(sumexp)
            nc.vector.scalar_tensor_tensor(
                out=vall[:, ic:ic + 1], in0=pdg[:], scalar=-SHIFT,
                in1=lnall[:, ic:ic + 1], op0=ALU.add, op1=ALU.subtract)

        nc.sync.dma_start(v_d[:], vall[:])

    nc.compile()
    return nc


_built = None


def _get_program():
    global _built
    if _built is None:
        _built = _build_program()
    return _built


def make_in_maps(inputs):
    import ml_dtypes
    BF = ml_dtypes.bfloat16
    f = lambda x: np.ascontiguousarray(np.asarray(x, dtype=np.float32))
    bf = lambda x: np.ascontiguousarray(np.asarray(x, np.float32).astype(BF))

    zw = np.concatenate([f(inputs['zw_0']), f(inputs['zw_1'])], axis=0)
    zwT = np.ascontiguousarray(zw.T)
    b0 = f(inputs['Ww0_b']) + f(inputs['Wk_b'])
    b1 = f(inputs['Ww1_b']) + f(inputs['Wk_b'])

    a1wB = np.zeros((128, 64), np.float32)
    a1wB[0:64, 0:32] = f(inputs['a0_1w']).T
    a1wB[64:128, 32:64] = f(inputs['a1_1w']).T
    a2wB = np.zeros((64, 2), np.float32)
    a2wB[0:32, 0:1] = f(inputs['a0_2w']).T
    a2wB[32:64, 1:2] = f(inputs['a1_2w']).T
    sel2 = np.zeros((2, 128), np.float32)
    sel2[0, 0:64] = 1.0
    sel2[1, 64:128] = 1.0
    linb2 = np.concatenate([f(inputs['lin0_b']), f(inputs['lin1_b'])])
    a1b2 = np.concatenate([f(inputs['a0_1b']), f(inputs['a1_1b'])])
    wk = f(inputs['Wk_w'])
    uw0 = np.hstack([wk, f(inputs['Ww0_w'])])   # [128,128]
    uw1 = np.hstack([wk, f(inputs['Ww1_w'])])

    base = {
        'zwTb': bf(zwT),
        'cT': np.ascontiguousarray(f(inputs['c']).T),
        'UW0': uw0,
        'UW1': uw1,
        'lwT0': bf(f(inputs['lin0_w']).T),
        'lwT1': bf(f(inputs['lin1_w']).T),
        'a1wB': bf(a1wB),
        'a2wB': bf(a2wB),
        'b0': bf(b0.reshape(128, 1)),
        'b1': bf(b1.reshape(128, 1)),
        'sel2': bf(sel2),
        'ones': bf(np.ones((128, 1), np.float32)),
        'linb2': linb2.reshape(128, 1),
        'a1b2': a1b2.reshape(64, 1),
    }
    in_maps = []
    for cid in range(N_CORES):
        g = cid // 4
        m = dict(base)
        zo = np.ascontiguousarray(zwT[:, cid * OWN:(cid + 1) * OWN])
        m['zwoT'] = zo
        m['zwoTb'] = bf(zo)
        m['UWo'] = uw0 if g == 0 else uw1
        m['b_own'] = bf((b0 if g == 0 else b1).reshape(128, 1))
        in_maps.append(m)
    return in_maps


def kernel(**inputs):
    nc = _get_program()
    in_maps = make_in_maps(inputs)
    res = run_bass_kernel_spmd(nc, in_maps, list(range(N_CORES)))
    tot = 0.0
    for r in res.results:
        tot += np.asarray(r['v'], dtype=np.float64).sum()
    return np.array(-(tot / B), dtype=np.float32)

